# revision 8
# baseline (speedup 1.0000x reference)
"""Multi-head causal attention (B=4, S=2048, D=1024, 16 heads) on 8 TRN2 cores.

Sharding: core c -> (batch b = c//2, head-group g = c%2). Each core computes
8 heads of one batch element end-to-end (QKV proj, causal softmax attention,
out-proj rows for its head slice). Host sums the two head-group partials per
batch and adds the output bias.

Per-core pipeline (all matmuls contraction-on-partitions, bf16 in / f32 psum):
  V first (din-major, fed by seq-chunked xT DMAs split across both HWDGE
  queues) so the PE starts ~1us in and the input load fully overlaps compute.
  QT/KT[dtile] = (x @ w)^T   [128p = 2 heads x 64, S]
  attention per (256-wide q-chunk qc, head-pair hp):
    ST[k,q] = KT.T @ QT into a [128,1024] psum (<=4 k-blocks per exp chunk)
    PT = exp(ST/8) bf16; tri-mask on diagonal 128-col blocks
    ctxT[q128, 65] += PT-block.T @ [V_h|1]   (N=65 matmuls: cost-model cheap;
      col 64 = softmax denominator), accumulated per (head, q128-sub) in one
      [128,512] psum bank; PV trails the exp by 2 chunks.
    normalize per-partition: ctxT[:, :64] * recip(ctxT[:, 64]) -> t_sb bf16
  t_sb tiles are PE-transposed (via identity) back to cxt[2-head-hd, q] layout.
  out[seq128, 512] = cxt.T @ ow streamed to DRAM per 128-row block.
  Transposes + out-proj pieces are interleaved as "filler" PE work between
  attention score chunks so the ACT engine (exp) always has scores in flight.
"""

import numpy as np
import ml_dtypes

B, S, D = 4, 2048, 1024
H_TOT = 16
HD = 64
NCORES = 8
GH = 8          # heads per core
GD = GH * HD    # 512: dout slice per core
NKB = S // 128  # 16 k-blocks
QW = 256        # q-chunk width
NQC = S // QW   # 8 q-chunks
BF16 = ml_dtypes.bfloat16

PACK_HEADS = True   # pack 2 heads' score matmuls into PE row groups

_cache = {}


def _build_body(tc, nc, mybir, xT, wq, wk, wv, ow, outp, dbg=None):
    from concourse.masks import make_upper_triangular, make_identity
    import contextlib

    dt = mybir.dt
    F = mybir.ActivationFunctionType

    pools = contextlib.ExitStack()
    tc_pool = lambda **kw: pools.enter_context(tc.tile_pool(**kw))

    singles = tc_pool(name="singles", bufs=1)
    pt_pool = tc_pool(name="pt", bufs=6)
    tsb_pool = tc_pool(name="tsb", bufs=8)
    small = tc_pool(name="small", bufs=4)
    ost_pool = tc_pool(name="ost", bufs=4)
    psum_st = tc_pool(name="psum_st", bufs=2, space="PSUM")
    psum_ctx = tc_pool(name="psum_ctx", bufs=2, space="PSUM")
    psum_mm = tc_pool(name="psum_mm", bufs=2, space="PSUM")

    # ---- persistent SBUF tensors ----
    xT_sb = [singles.tile([128, S], dt.bfloat16, name=f"xt{t}")
             for t in range(8)]
    wq_sb = [singles.tile([128, GD], dt.bfloat16, name=f"wq{t}")
             for t in range(8)]
    wk_sb = [singles.tile([128, GD], dt.bfloat16, name=f"wk{t}")
             for t in range(8)]
    wv_sb = [singles.tile([128, GD], dt.bfloat16, name=f"wv{t}")
             for t in range(8)]
    ow_sb = [singles.tile([128, D], dt.bfloat16, name=f"ow{t}")
             for t in range(4)]
    qt_sb = [singles.tile([128, S], dt.bfloat16, name=f"qt{t}")
             for t in range(4)]                              # 2 heads / dtile
    kt_sb = [singles.tile([128, S], dt.bfloat16, name=f"kt{t}")
             for t in range(4)]
    vo_sb = [singles.tile([128, GH, 65], dt.bfloat16, name=f"vo{t}")
             for t in range(NKB)]                            # [V_h | ones]
    cxt_sb = [singles.tile([128, S], dt.bfloat16, name=f"cxt{t}")
              for t in range(4)]                             # ctx, 2 heads/tile
    tri = singles.tile([128, 128], dt.bfloat16)              # keep k<=q
    ident = singles.tile([128, 128], dt.bfloat16)

    make_upper_triangular(nc, tri, val=1.0, diag=True)
    make_identity(nc, ident)
    for t in range(NKB):
        nc.vector.memset(vo_sb[t][:, :, 64:65], 1.0)

    # ---- input DMAs, alternating the two HWDGE queues (SP=sync, ACT=scalar)
    # Arrival order: wv, xT by 512-col seq chunk (din-major within), wq, wk,
    # ow.  The V projection (emitted first, din-major) consumes exactly in
    # this order, so the PE starts after ~2 tiles and never waits long.
    xT_r = xT.ap().rearrange("(t p) s -> p t s", p=128)
    wq_r = wq.ap().rearrange("(t p) n -> p t n", p=128)
    wk_r = wk.ap().rearrange("(t p) n -> p t n", p=128)
    wv_r = wv.ap().rearrange("(t p) n -> p t n", p=128)
    ow_r = ow.ap().rearrange("(t p) n -> p t n", p=128)

    qs = [nc.sync, nc.scalar]
    qi = 0

    def dma(out, in_):
        nonlocal qi
        qs[qi % 2].dma_start(out=out, in_=in_)
        qi += 1

    for din in range(8):
        dma(wv_sb[din], wv_r[:, din, :])
    for q4 in range(4):
        for din in range(8):
            dma(xT_sb[din][:, q4 * 512:(q4 + 1) * 512],
                xT_r[:, din, q4 * 512:(q4 + 1) * 512])
    for din in range(8):
        dma(wq_sb[din], wq_r[:, din, :])
    for din in range(8):
        dma(wk_sb[din], wk_r[:, din, :])
    for t in range(4):
        dma(ow_sb[t], ow_r[:, t, :])

    def emit_v_quarter(q4):
        """V proj for seq blocks 4*q4..4*q4+3, din-major so each (wv,xT-chunk)
        DMA pair unblocks the next 4 matmuls."""
        pst = [psum_st.tile([128, 1024], dt.float32, name="stp")
               for _ in range(2)]
        pss = [pst[0][:, 0:512], pst[0][:, 512:1024],
               pst[1][:, 0:512], pst[1][:, 512:1024]]
        for din in range(8):
            for i in range(4):
                st = 4 * q4 + i
                nc.tensor.matmul(
                    pss[i],
                    lhsT=xT_sb[din][:, st * 128:(st + 1) * 128],
                    rhs=wv_sb[din],
                    start=(din == 0),
                    stop=(din == 7),
                )
        for i in range(4):
            nc.vector.tensor_copy(
                out=vo_sb[4 * q4 + i][:, :, 0:64],
                in_=pss[i].rearrange("p (h d) -> p h d", h=GH),
            )

    def emit_proj_dtile(w_sb, t_sb, dtile):
        pst = [psum_st.tile([128, 1024], dt.float32, name="stp")
               for _ in range(2)]
        pss = [pst[0][:, 0:512], pst[0][:, 512:1024],
               pst[1][:, 0:512], pst[1][:, 512:1024]]
        for din in range(8):
            for c in range(4):
                nc.tensor.matmul(
                    pss[c],
                    lhsT=w_sb[din][:, dtile * 128:(dtile + 1) * 128],
                    rhs=xT_sb[din][:, c * 512:(c + 1) * 512],
                    start=(din == 0),
                    stop=(din == 7),
                )
        for c in range(4):
            nc.vector.tensor_copy(
                out=t_sb[dtile][:, c * 512:(c + 1) * 512], in_=pss[c])

    # ---- filler machinery: small PE work units interleaved into attention
    # score chunks so the exp (ACT) pipeline never starves the PE.
    fillers = []        # list of zero-arg emit functions
    tr_done = [0] * NQC  # transposes emitted per qc

    def emit_filler():
        if fillers:
            fillers.pop(0)()

    def emit_transpose(qc, hp):
        """Transpose the two normalized t_sb tiles of (qc, hp) into
        cxt_sb[hp][:, qc*256 : (qc+1)*256]."""
        tps = psum_mm.tile([128, 256], dt.bfloat16, name="mmps")
        ts0, ts1 = tsb_tiles[(qc, hp)]
        nc.tensor.transpose(tps[:, 0:128], ts0, ident)
        nc.tensor.transpose(tps[:, 128:256], ts1, ident)
        nc.vector.tensor_copy(
            out=cxt_sb[hp][:, qc * QW:(qc + 1) * QW], in_=tps)
        del tsb_tiles[(qc, hp)]
        tr_done[qc] += 1
        if tr_done[qc] == 4:
            for sq in (2 * qc, 2 * qc + 1):
                for oc in range(2):
                    fillers.append(
                        lambda sq=sq, oc=oc: emit_outproj_piece(sq, oc))

    def emit_outproj_piece(sq, oc):
        ps = psum_mm.tile([128, 512], dt.float32, name="mmps")
        for dvt in range(4):
            nc.tensor.matmul(
                ps,
                lhsT=cxt_sb[dvt][:, sq * 128:(sq + 1) * 128],
                rhs=ow_sb[dvt][:, oc * 512:(oc + 1) * 512],
                start=(dvt == 0),
                stop=(dvt == 3),
            )
        ost = ost_pool.tile([128, 512], dt.float32, name="ost")
        nc.vector.tensor_copy(out=ost, in_=ps)
        nc.sync.dma_start(
            out=outp.ap()[sq * 128:(sq + 1) * 128,
                          oc * 512:(oc + 1) * 512],
            in_=ost,
        )

    tsb_tiles = {}

    def emit_attn(qc, hp):
        """256-wide q chunk qc for heads h0=2*hp (PE rows 0:64) and h1=2*hp+1
        (rows 64:128).  Scores go into [128,1024] psums covering up to 4
        k-blocks; exp covers the whole psum; PV (N=65) trails by 2 chunks."""
        nkb = 2 * qc + 2
        q0 = QW * qc
        ctx_ps = psum_ctx.tile([128, 512], dt.float32, name="ctx")
        # ctxT blocks (head, j) at col 65*(2*head+j); j = q 128-sub-block.
        # PSUM "pending zero" semantics: ONE start=True marks the whole 2KB
        # bank pending, so each block's first write auto-zeroes; a second
        # start=True would wipe sibling blocks' partial sums.
        first_pv = [True]
        pend = []

        def emit_pv(half, kbs, offs, ns, pt):
            for kb, off, n in zip(kbs, offs, ns):
                qoff = QW - n           # first valid q-col within the chunk
                for j in range(2):
                    if 128 * j < qoff:
                        continue        # sub-block entirely above diagonal
                    seg = off + 128 * j - qoff
                    blk = 65 * (2 * half + j)
                    last_kb = min(nkb - 1, 2 * qc + j)
                    nc.tensor.matmul(
                        ctx_ps[:, blk:blk + 65],
                        lhsT=pt[:, seg:seg + 128],
                        rhs=vo_sb[kb][:, 2 * hp + half, :],
                        start=first_pv[0],
                        stop=(half == 1 and kb == last_kb == 2 * qc + 1),
                        skip_group_check=True,
                    )
                    first_pv[0] = False

        for c0 in range(0, nkb, 4):
            kbs = list(range(c0, min(c0 + 4, nkb)))
            ns = [QW - max(0, kb * 128 - q0) for kb in kbs]
            offs = list(np.cumsum([0] + ns[:-1]))
            ntot = offs[-1] + ns[-1]
            for half in range(2):
                p0 = half * 64
                stp = psum_st.tile([128, 1024], dt.float32, name="stp")
                for kb, off, n in zip(kbs, offs, ns):
                    nc.tensor.matmul(
                        stp[:, off:off + n],
                        lhsT=kt_sb[hp][p0:p0 + 64, kb * 128:(kb + 1) * 128],
                        rhs=qt_sb[hp][p0:p0 + 64, q0 + QW - n:q0 + QW],
                        start=True,
                        stop=True,
                        tile_position=(p0, 0) if PACK_HEADS else None,
                    )
                emit_filler()
                pt = pt_pool.tile([128, 1024], dt.bfloat16, name="pt")
                nc.scalar.activation(
                    out=pt[:, :ntot], in_=stp[:, :ntot], func=F.Exp,
                    scale=0.125)
                for kb, off, n in zip(kbs, offs, ns):
                    # diagonal blocks: kb == 2*qc + j at segment col 128j-qoff
                    qoff = QW - n
                    if kb == 2 * qc:            # j=0 diag (n==256 case)
                        if qoff == 0:
                            nc.vector.tensor_mul(
                                pt[:, off:off + 128], pt[:, off:off + 128],
                                tri)
                    if kb == 2 * qc + 1:        # j=1 diag (n==128 case)
                        nc.vector.tensor_mul(
                            pt[:, off:off + 128], pt[:, off:off + 128], tri)
                pend.append((half, kbs, offs, ns, pt))
                if len(pend) > 4:
                    emit_pv(*pend.pop(0))
        for p in pend:
            emit_pv(*p)
        # normalize: per-partition multiply by 1/denominator (psum col 64+65b)
        recip = small.tile([128, 4], dt.float32, name="recip")
        for b in range(4):
            nc.vector.reciprocal(
                out=recip[:, b:b + 1], in_=ctx_ps[:, 65 * b + 64:65 * b + 65])
        ts = [tsb_pool.tile([128, 128], dt.bfloat16, name="tsb")
              for _ in range(2)]
        for half in range(2):
            for j in range(2):
                blk = 65 * (2 * half + j)
                nc.vector.tensor_scalar_mul(
                    out=ts[j][:, half * 64:(half + 1) * 64],
                    in0=ctx_ps[:, blk:blk + 64],
                    scalar1=recip[:, 2 * half + j:2 * half + j + 1],
                )
        tsb_tiles[(qc, hp)] = ts
        fillers.append(lambda: emit_transpose(qc, hp))

    # ---- emission schedule ----
    for q4 in range(4):
        emit_v_quarter(q4)
    for dtile in range(4):
        emit_proj_dtile(wq_sb, qt_sb, dtile)
        emit_proj_dtile(wk_sb, kt_sb, dtile)
        if dtile > 0:   # one-dtile delay so the qt/kt copies are done
            emit_attn(0, dtile - 1)
            emit_attn(1, dtile - 1)
    emit_attn(0, 3)
    emit_attn(1, 3)
    for qc in range(2, NQC):
        for hp in range(4):
            emit_attn(qc, hp)
    while fillers:
        emit_filler()

    if dbg is not None:
        for t in range(4):
            nc.sync.dma_start(out=dbg["qt"].ap()[t], in_=qt_sb[t])
            nc.sync.dma_start(out=dbg["kt"].ap()[t], in_=kt_sb[t])
            nc.sync.dma_start(out=dbg["cxt"].ap()[t], in_=cxt_sb[t])
        for t in range(16):
            nc.sync.dma_start(out=dbg["vo"].ap()[t], in_=vo_sb[t])

    return pools


def _build_nc():
    import concourse.tile as tile
    from concourse import bacc, mybir

    dt = mybir.dt
    nc = bacc.Bacc("TRN2", target_bir_lowering=False, debug=False,
                   num_devices=NCORES)
    xT = nc.dram_tensor("xt", [D, S], dt.bfloat16, kind="ExternalInput")
    wq = nc.dram_tensor("wq", [D, GD], dt.bfloat16, kind="ExternalInput")
    wk = nc.dram_tensor("wk", [D, GD], dt.bfloat16, kind="ExternalInput")
    wv = nc.dram_tensor("wv", [D, GD], dt.bfloat16, kind="ExternalInput")
    ow = nc.dram_tensor("ow", [GD, D], dt.bfloat16, kind="ExternalInput")
    outp = nc.dram_tensor("outp", [S, D], dt.float32, kind="ExternalOutput")

    with tile.TileContext(nc) as tc:
        pools = _build_body(tc, nc, mybir, xT, wq, wk, wv, ow, outp)
        pools.close()
    nc.compile()
    return nc


LAST_RESULTS = None


def kernel(batch, w_query, w_key, w_value, out_w, out_b):
    global LAST_RESULTS
    import os
    from concourse import bass_utils

    try:  # BASS_TRACE needs the axon NTFF hook; without it the run crashes
        from antenv.axon_hooks import get_axon_ntff_profile_hook  # noqa: F401
    except ImportError:
        os.environ.setdefault("BASS_NEVER_TRACE", "1")

    batch = np.asarray(batch, dtype=np.float32)
    w_query = np.asarray(w_query, dtype=np.float32)
    w_key = np.asarray(w_key, dtype=np.float32)
    w_value = np.asarray(w_value, dtype=np.float32)
    out_w = np.asarray(out_w, dtype=np.float32)
    out_b = np.asarray(out_b, dtype=np.float32)

    if "nc" not in _cache:
        _cache["nc"] = _build_nc()
    nc = _cache["nc"]

    xts = [np.ascontiguousarray(batch[b].T).astype(BF16) for b in range(B)]
    slc = [slice(g * GD, (g + 1) * GD) for g in range(2)]
    wqs = [np.ascontiguousarray(w_query[:, s]).astype(BF16) for s in slc]
    wks = [np.ascontiguousarray(w_key[:, s]).astype(BF16) for s in slc]
    wvs = [np.ascontiguousarray(w_value[:, s]).astype(BF16) for s in slc]
    ows = [np.ascontiguousarray(out_w[s, :]).astype(BF16) for s in slc]
    in_maps = []
    for c in range(NCORES):
        b, g = divmod(c, 2)
        in_maps.append({
            "xt": xts[b], "wq": wqs[g], "wk": wks[g],
            "wv": wvs[g], "ow": ows[g],
        })

    res = bass_utils.run_bass_kernel_spmd(
        nc, in_maps, core_ids=list(range(NCORES)),
    )
    LAST_RESULTS = res

    out = np.empty((B, S, D), np.float32)
    for b in range(B):
        out[b] = res.results[2 * b]["outp"] + res.results[2 * b + 1]["outp"] \
            + out_b[None, :]
    return out


# revision 9
# speedup vs baseline: 1.1202x; 1.1202x over previous
"""Multi-head causal attention (B=4, S=2048, D=1024, 16 heads) on 8 TRN2 cores.

Sharding: core c -> (batch b = c//2, head-group g = c%2). Each core computes
8 heads of one batch element end-to-end (QKV proj, causal softmax attention,
out-proj rows for its head slice). Host sums the two head-group partials per
batch and adds the output bias.

Per-core pipeline (all matmuls contraction-on-partitions, bf16 in / f32 psum):
  V first (quarter 0 din-major so each (wv,xT-chunk) DMA pair unblocks work;
  later quarters group-major so psum->sbuf copies overlap the remaining
  matmuls), then QT/KT[dtile] = (x @ w)^T with attention interleaved.
  attention per (256-wide q-chunk qc, head-pair hp):
    ST[k,q] = KT.T @ QT into a [128,1024] psum (<=4 k-blocks per exp chunk)
    PT = exp(ST/8) bf16; tri-mask on diagonal 128-col blocks
    ctxT[q128, 65] += PT-block.T @ [V_h|1]   (N=65 matmuls: cost-model cheap;
      col 64 = softmax denominator). One start=True per ctx psum bank (PSUM
      pending-zero covers every block's first write; a second start would
      wipe sibling blocks).  PV trails the exp by 2 chunk-halves ACROSS
      attention units so the PE never sits behind the ACT engine.
    normalize per-partition: ctxT[:, :64] * recip(ctxT[:, 64]) -> t_sb bf16
  t_sb tiles are PE-transposed (via identity) back to cxt[2-head-hd, q].
  out[seq128, 512] = cxt.T @ ow streamed to DRAM per 128-row block.
  Transposes + out-proj pieces are "fillers" popped between score chunks;
  the late phase runs qc descending so out-proj work for early rows is
  available to fill the exp-heavy qc=7/6 units.
"""

import numpy as np
import ml_dtypes

B, S, D = 4, 2048, 1024
H_TOT = 16
HD = 64
NCORES = 8
GH = 8          # heads per core
GD = GH * HD    # 512: dout slice per core
NKB = S // 128  # 16 k-blocks
QW = 256        # q-chunk width
NQC = S // QW   # 8 q-chunks
BF16 = ml_dtypes.bfloat16

PACK_HEADS = True   # pack 2 heads' score matmuls into PE row groups
PV_DEPTH = 2        # chunk-halves the PV trails behind the exp

_cache = {}


def _build_body(tc, nc, mybir, xT, wq, wk, wv, ow, outp, dbg=None):
    from concourse.masks import make_upper_triangular, make_identity
    import contextlib

    dt = mybir.dt
    F = mybir.ActivationFunctionType

    pools = contextlib.ExitStack()
    tc_pool = lambda **kw: pools.enter_context(tc.tile_pool(**kw))

    singles = tc_pool(name="singles", bufs=1)
    pt_pool = tc_pool(name="pt", bufs=6)
    tsb_pool = tc_pool(name="tsb", bufs=8)
    small = tc_pool(name="small", bufs=4)
    ost_pool = tc_pool(name="ost", bufs=4)
    psum_st = tc_pool(name="psum_st", bufs=2, space="PSUM")
    psum_ctx = tc_pool(name="psum_ctx", bufs=2, space="PSUM")
    psum_mm = tc_pool(name="psum_mm", bufs=2, space="PSUM")

    # ---- persistent SBUF tensors ----
    xT_sb = [singles.tile([128, S], dt.bfloat16, name=f"xt{t}")
             for t in range(8)]
    wq_sb = [singles.tile([128, GD], dt.bfloat16, name=f"wq{t}")
             for t in range(8)]
    wk_sb = [singles.tile([128, GD], dt.bfloat16, name=f"wk{t}")
             for t in range(8)]
    wv_sb = [singles.tile([128, GD], dt.bfloat16, name=f"wv{t}")
             for t in range(8)]
    ow_sb = [singles.tile([128, D], dt.bfloat16, name=f"ow{t}")
             for t in range(4)]
    qt_sb = [singles.tile([128, S], dt.bfloat16, name=f"qt{t}")
             for t in range(4)]                              # 2 heads / dtile
    kt_sb = [singles.tile([128, S], dt.bfloat16, name=f"kt{t}")
             for t in range(4)]
    vo_sb = [singles.tile([128, GH, 65], dt.bfloat16, name=f"vo{t}")
             for t in range(NKB)]                            # [V_h | ones]
    cxt_sb = [singles.tile([128, S], dt.bfloat16, name=f"cxt{t}")
              for t in range(4)]                             # ctx, 2 heads/tile
    tri = singles.tile([128, 128], dt.bfloat16)              # keep k<=q
    ident = singles.tile([128, 128], dt.bfloat16)

    make_upper_triangular(nc, tri, val=1.0, diag=True)
    make_identity(nc, ident)
    for t in range(NKB):
        nc.vector.memset(vo_sb[t][:, :, 64:65], 1.0)

    # ---- input DMAs, alternating the two HWDGE queues (SP=sync, ACT=scalar)
    xT_r = xT.ap().rearrange("(t p) s -> p t s", p=128)
    wq_r = wq.ap().rearrange("(t p) n -> p t n", p=128)
    wk_r = wk.ap().rearrange("(t p) n -> p t n", p=128)
    wv_r = wv.ap().rearrange("(t p) n -> p t n", p=128)
    ow_r = ow.ap().rearrange("(t p) n -> p t n", p=128)

    qs = [nc.sync, nc.scalar]
    qi = 0

    def dma(out, in_):
        nonlocal qi
        qs[qi % 2].dma_start(out=out, in_=in_)
        qi += 1

    for din in range(8):        # paired so V quarter 0 unblocks din by din
        dma(wv_sb[din], wv_r[:, din, :])
        dma(xT_sb[din][:, 0:512], xT_r[:, din, 0:512])
    for q4 in range(1, 4):
        for din in range(8):
            dma(xT_sb[din][:, q4 * 512:(q4 + 1) * 512],
                xT_r[:, din, q4 * 512:(q4 + 1) * 512])
    for din in range(8):
        dma(wq_sb[din], wq_r[:, din, :])
    for din in range(8):
        dma(wk_sb[din], wk_r[:, din, :])
    for t in range(4):
        dma(ow_sb[t], ow_r[:, t, :])

    def emit_v_quarter(q4):
        """V proj for seq blocks 4*q4..4*q4+3.  Quarter 0 is DMA-paced:
        din-major so each arriving (wv,xT) pair feeds 4 matmuls.  Later
        quarters are group-major so each group's copy overlaps the rest."""
        pst = [psum_st.tile([128, 1024], dt.float32, name="stp")
               for _ in range(2)]
        pss = [pst[0][:, 0:512], pst[0][:, 512:1024],
               pst[1][:, 0:512], pst[1][:, 512:1024]]

        def mm(i, din):
            nc.tensor.matmul(
                pss[i],
                lhsT=xT_sb[din][:, (4 * q4 + i) * 128:(4 * q4 + i + 1) * 128],
                rhs=wv_sb[din],
                start=(din == 0),
                stop=(din == 7),
            )

        def cp(i):
            nc.vector.tensor_copy(
                out=vo_sb[4 * q4 + i][:, :, 0:64],
                in_=pss[i].rearrange("p (h d) -> p h d", h=GH),
            )

        if q4 == 0:
            for din in range(8):
                for i in range(4):
                    mm(i, din)
            for i in range(4):
                cp(i)
        else:
            for i in range(4):
                for din in range(8):
                    mm(i, din)
                cp(i)

    def emit_proj_dtile(w_sb, t_sb, dtile):
        """Q/K projection, group-major: copy chunk c while c+1 computes."""
        pst = [psum_st.tile([128, 1024], dt.float32, name="stp")
               for _ in range(2)]
        pss = [pst[0][:, 0:512], pst[0][:, 512:1024],
               pst[1][:, 0:512], pst[1][:, 512:1024]]
        for c in range(4):
            for din in range(8):
                nc.tensor.matmul(
                    pss[c],
                    lhsT=w_sb[din][:, dtile * 128:(dtile + 1) * 128],
                    rhs=xT_sb[din][:, c * 512:(c + 1) * 512],
                    start=(din == 0),
                    stop=(din == 7),
                )
            nc.vector.tensor_copy(
                out=t_sb[dtile][:, c * 512:(c + 1) * 512], in_=pss[c])

    # ---- filler machinery: small PE work units popped between score chunks
    slot = [0]          # global chunk-half counter
    fillers = []        # [ready_slot, fn] entries, FIFO among ready
    tr_done = [0] * NQC

    def emit_filler():
        for i, (rdy, fn) in enumerate(fillers):
            if rdy <= slot[0]:
                fillers.pop(i)
                fn()
                return

    def flush_fillers():
        while fillers:
            rdy, fn = fillers.pop(0)
            fn()

    def emit_transpose(qc, hp):
        tps = psum_mm.tile([128, 256], dt.bfloat16, name="mmps")
        ts0, ts1 = tsb_tiles.pop((qc, hp))
        nc.tensor.transpose(tps[:, 0:128], ts0, ident)
        nc.tensor.transpose(tps[:, 128:256], ts1, ident)
        nc.vector.tensor_copy(
            out=cxt_sb[hp][:, qc * QW:(qc + 1) * QW], in_=tps)
        tr_done[qc] += 1
        if tr_done[qc] == 4:
            for sq in (2 * qc, 2 * qc + 1):
                for oc in range(2):
                    fillers.append(
                        [slot[0] + 2,
                         lambda sq=sq, oc=oc: emit_outproj_piece(sq, oc)])

    def emit_outproj_piece(sq, oc):
        ps = psum_mm.tile([128, 512], dt.float32, name="mmps")
        for dvt in range(4):
            nc.tensor.matmul(
                ps,
                lhsT=cxt_sb[dvt][:, sq * 128:(sq + 1) * 128],
                rhs=ow_sb[dvt][:, oc * 512:(oc + 1) * 512],
                start=(dvt == 0),
                stop=(dvt == 3),
            )
        ost = ost_pool.tile([128, 512], dt.float32, name="ost")
        nc.vector.tensor_copy(out=ost, in_=ps)
        nc.sync.dma_start(
            out=outp.ap()[sq * 128:(sq + 1) * 128,
                          oc * 512:(oc + 1) * 512],
            in_=ost,
        )

    tsb_tiles = {}
    pend = []           # cross-unit PV trail: closures

    def pop_pend():
        if pend:
            pend.pop(0)()

    def emit_attn(qc, hp):
        """256-wide q chunk qc for heads h0=2*hp (PE rows 0:64) and h1=2*hp+1
        (rows 64:128)."""
        nkb = 2 * qc + 2
        q0 = QW * qc
        ctx_ps = psum_ctx.tile([128, 512], dt.float32, name="ctx")
        first_pv = [True]
        n_halves = 2 * ((nkb + 3) // 4)
        emitted = [0]

        def emit_pv(half, kbs, offs, ns, pt):
            for kb, off, n in zip(kbs, offs, ns):
                qoff = QW - n           # first valid q-col within the chunk
                for j in range(2):
                    if 128 * j < qoff:
                        continue        # sub-block entirely above diagonal
                    seg = off + 128 * j - qoff
                    blk = 65 * (2 * half + j)
                    last_kb = min(nkb - 1, 2 * qc + j)
                    nc.tensor.matmul(
                        ctx_ps[:, blk:blk + 65],
                        lhsT=pt[:, seg:seg + 128],
                        rhs=vo_sb[kb][:, 2 * hp + half, :],
                        start=first_pv[0],
                        stop=(half == 1 and kb == last_kb == 2 * qc + 1),
                        skip_group_check=True,
                    )
                    first_pv[0] = False
            emitted[0] += 1
            if emitted[0] == n_halves:
                emit_norm()

        def emit_norm():
            recip = small.tile([128, 4], dt.float32, name="recip")
            for b in range(4):
                nc.vector.reciprocal(
                    out=recip[:, b:b + 1],
                    in_=ctx_ps[:, 65 * b + 64:65 * b + 65])
            ts = [tsb_pool.tile([128, 128], dt.bfloat16, name="tsb")
                  for _ in range(2)]
            for half in range(2):
                for j in range(2):
                    blk = 65 * (2 * half + j)
                    nc.vector.tensor_scalar_mul(
                        out=ts[j][:, half * 64:(half + 1) * 64],
                        in0=ctx_ps[:, blk:blk + 64],
                        scalar1=recip[:, 2 * half + j:2 * half + j + 1],
                    )
            tsb_tiles[(qc, hp)] = ts
            fillers.append([slot[0] + 2, lambda: emit_transpose(qc, hp)])

        for c0 in range(0, nkb, 4):
            kbs = list(range(c0, min(c0 + 4, nkb)))
            ns = [QW - max(0, kb * 128 - q0) for kb in kbs]
            offs = [int(v) for v in np.cumsum([0] + ns[:-1])]
            ntot = offs[-1] + ns[-1]
            for half in range(2):
                p0 = half * 64
                stp = psum_st.tile([128, 1024], dt.float32, name="stp")
                for kb, off, n in zip(kbs, offs, ns):
                    nc.tensor.matmul(
                        stp[:, off:off + n],
                        lhsT=kt_sb[hp][p0:p0 + 64, kb * 128:(kb + 1) * 128],
                        rhs=qt_sb[hp][p0:p0 + 64, q0 + QW - n:q0 + QW],
                        start=True,
                        stop=True,
                        tile_position=(p0, 0) if PACK_HEADS else None,
                    )
                slot[0] += 1
                emit_filler()
                pt = pt_pool.tile([128, 1024], dt.bfloat16, name="pt")
                nc.scalar.activation(
                    out=pt[:, :ntot], in_=stp[:, :ntot], func=F.Exp,
                    scale=0.125)
                for kb, off, n in zip(kbs, offs, ns):
                    qoff = QW - n
                    if kb == 2 * qc and qoff == 0:      # j=0 diagonal block
                        nc.vector.tensor_mul(
                            pt[:, off:off + 128], pt[:, off:off + 128], tri)
                    if kb == 2 * qc + 1:                # j=1 diagonal block
                        nc.vector.tensor_mul(
                            pt[:, off:off + 128], pt[:, off:off + 128], tri)
                pend.append(
                    lambda a=half, b=kbs, c=offs, d=ns, e=pt:
                    emit_pv(a, b, c, d, e))
                while len(pend) > PV_DEPTH:
                    pop_pend()

    # ---- emission schedule ----
    for q4 in range(4):
        emit_v_quarter(q4)
    for dtile in range(4):
        emit_proj_dtile(wq_sb, qt_sb, dtile)
        emit_proj_dtile(wk_sb, kt_sb, dtile)
        if dtile > 0:   # one-dtile delay so the qt/kt copies are done
            emit_attn(0, dtile - 1)
            emit_attn(1, dtile - 1)
    emit_attn(0, 3)
    emit_attn(1, 3)
    # late phase: qc descending so out-proj for low rows (already available)
    # fills the exp-heavy qc=7/6 units
    for qc in range(NQC - 1, 1, -1):
        for hp in range(4):
            emit_attn(qc, hp)
    while pend:
        pop_pend()
    flush_fillers()

    if dbg is not None:
        for t in range(4):
            nc.sync.dma_start(out=dbg["qt"].ap()[t], in_=qt_sb[t])
            nc.sync.dma_start(out=dbg["kt"].ap()[t], in_=kt_sb[t])
            nc.sync.dma_start(out=dbg["cxt"].ap()[t], in_=cxt_sb[t])
        for t in range(16):
            nc.sync.dma_start(out=dbg["vo"].ap()[t], in_=vo_sb[t])

    return pools


def _build_nc():
    import concourse.tile as tile
    from concourse import bacc, mybir

    dt = mybir.dt
    nc = bacc.Bacc("TRN2", target_bir_lowering=False, debug=False,
                   num_devices=NCORES)
    xT = nc.dram_tensor("xt", [D, S], dt.bfloat16, kind="ExternalInput")
    wq = nc.dram_tensor("wq", [D, GD], dt.bfloat16, kind="ExternalInput")
    wk = nc.dram_tensor("wk", [D, GD], dt.bfloat16, kind="ExternalInput")
    wv = nc.dram_tensor("wv", [D, GD], dt.bfloat16, kind="ExternalInput")
    ow = nc.dram_tensor("ow", [GD, D], dt.bfloat16, kind="ExternalInput")
    outp = nc.dram_tensor("outp", [S, D], dt.float32, kind="ExternalOutput")

    with tile.TileContext(nc) as tc:
        pools = _build_body(tc, nc, mybir, xT, wq, wk, wv, ow, outp)
        pools.close()
    nc.compile()
    return nc


LAST_RESULTS = None


def kernel(batch, w_query, w_key, w_value, out_w, out_b):
    global LAST_RESULTS
    import os
    from concourse import bass_utils

    try:  # BASS_TRACE needs the axon NTFF hook; without it the run crashes
        from antenv.axon_hooks import get_axon_ntff_profile_hook  # noqa: F401
    except ImportError:
        os.environ.setdefault("BASS_NEVER_TRACE", "1")

    batch = np.asarray(batch, dtype=np.float32)
    w_query = np.asarray(w_query, dtype=np.float32)
    w_key = np.asarray(w_key, dtype=np.float32)
    w_value = np.asarray(w_value, dtype=np.float32)
    out_w = np.asarray(out_w, dtype=np.float32)
    out_b = np.asarray(out_b, dtype=np.float32)

    if "nc" not in _cache:
        _cache["nc"] = _build_nc()
    nc = _cache["nc"]

    xts = [np.ascontiguousarray(batch[b].T).astype(BF16) for b in range(B)]
    slc = [slice(g * GD, (g + 1) * GD) for g in range(2)]
    wqs = [np.ascontiguousarray(w_query[:, s]).astype(BF16) for s in slc]
    wks = [np.ascontiguousarray(w_key[:, s]).astype(BF16) for s in slc]
    wvs = [np.ascontiguousarray(w_value[:, s]).astype(BF16) for s in slc]
    ows = [np.ascontiguousarray(out_w[s, :]).astype(BF16) for s in slc]
    in_maps = []
    for c in range(NCORES):
        b, g = divmod(c, 2)
        in_maps.append({
            "xt": xts[b], "wq": wqs[g], "wk": wks[g],
            "wv": wvs[g], "ow": ows[g],
        })

    res = bass_utils.run_bass_kernel_spmd(
        nc, in_maps, core_ids=list(range(NCORES)),
    )
    LAST_RESULTS = res

    out = np.empty((B, S, D), np.float32)
    for b in range(B):
        out[b] = res.results[2 * b]["outp"] + res.results[2 * b + 1]["outp"] \
            + out_b[None, :]
    return out
